# revision 10
# baseline (speedup 1.0000x reference)
"""Trainium2 Bass kernel for CFGSubASTExpressionCombiner (segment-softmax
attention over sub-ASTs grouped by PDG node).

Contract: kernel(**inputs) takes FULL unsharded numpy inputs, returns the
FULL [N_PDG, D] output. Internally shards PDG segments across 8 NeuronCores
(contiguous segment ranges, elements sorted by segment on host), replicates
ast_nodes_encodings + weights, and runs one SPMD Bass program.

Math (per segment s with element set E_s, all on device):
    q_s   = ast[root(s)]
    qk_s  = q_s @ (Wk.T * scale)        (scale folded into host-passed WkT)
    S[e,s]= x_e . qk_s                  (dense per 128-seg block, via PE)
    P     = exp(S) * [seg(e)==s]        (masked exp == softmax numerator)
    [U|Z] = P.T @ [X|1]                 (segment sums via PE, PSUM accum)
    out_s = (U_s / max(Z_s,eps)) @ Wv
No max-subtraction: scores are ~N(0,1) (|s|<~6), exp is safe in fp32, and
softmax is shift-invariant so results match the reference to fp32 rounding.
"""

import math

import numpy as np

import concourse.bass as bass
import concourse.bacc as bacc
import concourse.mybir as mybir
import concourse.tile as tile
from concourse.bass_utils import run_bass_kernel_spmd
from concourse.masks import make_identity

P = 128
D = 256
N_CORES = 8

# Full-problem constants (hardcoded per contract).
N_AST_FULL = 500000
N_PDG_FULL = 50000
SEGS_PER_CORE_FULL = N_PDG_FULL // N_CORES          # 6250
N_BLOCKS_FULL = math.ceil(SEGS_PER_CORE_FULL / P)   # 49
T_B_MIN = 12                                        # element tiles per block

f32 = mybir.dt.float32
i32 = mybir.dt.int32


def _build_nc(n_ast, n_blocks, t_b):
    """One SPMD NeuronCore program. Sizes fixed at build time."""
    seg_slots = n_blocks * P
    e_slots = n_blocks * t_b * P

    nc = bacc.Bacc()
    ast = nc.declare_dram_parameter("ast", [n_ast, D], f32, isOutput=False)
    wkt = nc.declare_dram_parameter("wkt", [D, D], f32, isOutput=False)
    wv = nc.declare_dram_parameter("wv", [D, D], f32, isOutput=False)
    gidx = nc.declare_dram_parameter("gidx", [e_slots], i32, isOutput=False)
    slid = nc.declare_dram_parameter("slid", [e_slots], f32, isOutput=False)
    root = nc.declare_dram_parameter("root", [seg_slots], i32, isOutput=False)
    out = nc.declare_dram_parameter("out", [seg_slots, D], f32, isOutput=True)

    EXP = mybir.ActivationFunctionType.Exp
    COPY = mybir.ActivationFunctionType.Copy

    with tile.TileContext(nc) as tc:
        with (
            tc.tile_pool(name="const", bufs=1) as cpool,
            tc.tile_pool(name="blk", bufs=2) as bpool,
            tc.tile_pool(name="xp", bufs=3) as xpool,
            tc.tile_pool(name="pt", bufs=2, space="PSUM") as pt,
            tc.tile_pool(name="pu", bufs=2, space="PSUM") as pu,
        ):
            # Resident constants: Wk.T (pre-scaled) and Wv as two 128-row
            # K-chunks side by side; identity for PE transpose; iota row.
            wk2 = cpool.tile([P, 2 * D], f32)
            nc.sync.dma_start(out=wk2[:, 0:D], in_=wkt[0:P, :])
            nc.sync.dma_start(out=wk2[:, D : 2 * D], in_=wkt[P : 2 * P, :])
            wv2 = cpool.tile([P, 2 * D], f32)
            nc.sync.dma_start(out=wv2[:, 0:D], in_=wv[0:P, :])
            nc.sync.dma_start(out=wv2[:, D : 2 * D], in_=wv[P : 2 * P, :])
            ident = cpool.tile([P, P], f32)
            make_identity(nc, ident[:])
            iota_i = cpool.tile([P, P], i32)
            nc.gpsimd.iota(iota_i[:], pattern=[[1, P]], base=0, channel_multiplier=0)
            iota_f = cpool.tile([P, P], f32)
            nc.vector.tensor_copy(iota_f[:], iota_i[:])

            # All index arrays resident in SBUF, one DMA each: column c of
            # gx_all/sl_all is element-tile c = b*t_b + t; column b of
            # root_all is segment block b.
            n_cols = n_blocks * t_b
            gx_all = cpool.tile([P, n_cols], i32)
            nc.gpsimd.dma_start(
                out=gx_all[:], in_=gidx[:].rearrange("(p c) -> p c", c=n_cols)
            )
            sl_all = cpool.tile([P, n_cols], f32)
            nc.gpsimd.dma_start(
                out=sl_all[:], in_=slid[:].rearrange("(p c) -> p c", c=n_cols)
            )
            root_all = cpool.tile([P, n_blocks], i32)
            nc.gpsimd.dma_start(
                out=root_all[:], in_=root[:].rearrange("(p b) -> p b", b=n_blocks)
            )

            for b in range(n_blocks):
                # ---- segment side: q rows -> qk^T (d on partitions) ----
                q = bpool.tile([P, D], f32)
                nc.gpsimd.indirect_dma_start(
                    out=q[:],
                    out_offset=None,
                    in_=ast[:],
                    in_offset=bass.IndirectOffsetOnAxis(ap=root_all[:, b : b + 1], axis=0),
                )
                qT_ps = pt.tile([P, D], f32, tag="tr")
                nc.tensor.transpose(qT_ps[:, 0:P], q[:, 0:P], ident[:])
                nc.tensor.transpose(qT_ps[:, P:D], q[:, P:D], ident[:])
                qT = bpool.tile([P, D], f32)
                nc.any.tensor_copy(qT[:], qT_ps[:])

                qkT_ps = pt.tile([P, D], f32, tag="mm")
                for m in range(2):
                    for k in range(2):
                        nc.tensor.matmul(
                            qkT_ps[:, m * P : (m + 1) * P],
                            lhsT=wk2[:, k * D + m * P : k * D + (m + 1) * P],
                            rhs=qT[:, k * P : (k + 1) * P],
                            start=(k == 0),
                            stop=(k == 1),
                        )
                qkT = bpool.tile([P, D], f32)
                nc.any.tensor_copy(qkT[:], qkT_ps[:])

                # ---- element side: accumulate [U | Z] over t_b tiles ----
                u_ps = pu.tile([P, D + 1], f32, tag="u")
                for t in range(t_b):
                    c = b * t_b + t
                    x = xpool.tile([P, D + 1], f32)
                    nc.vector.memset(x[:, D : D + 1], 1.0)
                    nc.gpsimd.indirect_dma_start(
                        out=x[:, 0:D],
                        out_offset=None,
                        in_=ast[:],
                        in_offset=bass.IndirectOffsetOnAxis(
                            ap=gx_all[:, c : c + 1], axis=0
                        ),
                    )
                    xT_ps = pt.tile([P, D], f32, tag="tr")
                    nc.tensor.transpose(xT_ps[:, 0:P], x[:, 0:P], ident[:])
                    nc.tensor.transpose(xT_ps[:, P:D], x[:, P:D], ident[:])
                    xT = xpool.tile([P, D], f32)
                    nc.any.tensor_copy(xT[:], xT_ps[:])

                    s_ps = pt.tile([P, P], f32, tag="s")
                    for k in range(2):
                        nc.tensor.matmul(
                            s_ps[:],
                            lhsT=xT[:, k * P : (k + 1) * P],
                            rhs=qkT[:, k * P : (k + 1) * P],
                            start=(k == 0),
                            stop=(k == 1),
                        )
                    ptil = xpool.tile([P, P], f32)
                    nc.scalar.activation(ptil[:], s_ps[:], EXP)
                    ind = xpool.tile([P, P], f32)
                    nc.vector.tensor_scalar(
                        out=ind[:],
                        in0=iota_f[:],
                        scalar1=sl_all[:, c : c + 1],
                        scalar2=None,
                        op0=mybir.AluOpType.is_equal,
                    )
                    pmat = xpool.tile([P, P], f32)
                    nc.vector.tensor_mul(pmat[:], ptil[:], ind[:])
                    nc.tensor.matmul(
                        u_ps[:],
                        lhsT=pmat[:],
                        rhs=x[:],
                        start=(t == 0),
                        stop=(t == t_b - 1),
                    )

                # ---- finalize block: U/Z @ Wv ----
                z = bpool.tile([P, 1], f32)
                nc.vector.tensor_scalar_max(z[:], u_ps[:, D : D + 1], 1e-30)
                rz = bpool.tile([P, 1], f32)
                nc.vector.reciprocal(rz[:], z[:])
                up = bpool.tile([P, D], f32)
                nc.scalar.activation(up[:], u_ps[:, 0:D], COPY, scale=rz[:, :1])
                upT_ps = pt.tile([P, D], f32, tag="tr")
                nc.tensor.transpose(upT_ps[:, 0:P], up[:, 0:P], ident[:])
                nc.tensor.transpose(upT_ps[:, P:D], up[:, P:D], ident[:])
                upT = bpool.tile([P, D], f32)
                nc.any.tensor_copy(upT[:], upT_ps[:])
                f_ps = pt.tile([P, D], f32, tag="mm")
                for k in range(2):
                    nc.tensor.matmul(
                        f_ps[:],
                        lhsT=upT[:, k * P : (k + 1) * P],
                        rhs=wv2[:, k * D : (k + 1) * D],
                        start=(k == 0),
                        stop=(k == 1),
                    )
                o = bpool.tile([P, D], f32)
                nc.any.tensor_copy(o[:], f_ps[:])
                nc.sync.dma_start(out=out[b * P : (b + 1) * P, :], in_=o[:])
    nc.finalize()
    return nc


_NC_CACHE = {}


def _get_nc(n_ast, n_blocks, t_b):
    key = (n_ast, n_blocks, t_b)
    if key not in _NC_CACHE:
        _NC_CACHE[key] = _build_nc(n_ast, n_blocks, t_b)
    return _NC_CACHE[key]


def _prepare_core_inputs(
    c, n_pdg, segs_per_core, n_blocks, t_b, cum, seg_sorted, gid_sorted, root_full
):
    """Padded per-core index arrays for core c's contiguous segment range."""
    seg_slots = n_blocks * P
    e_slots = n_blocks * t_b * P
    s0 = c * segs_per_core
    s1 = min(s0 + segs_per_core, n_pdg)

    # Linear (tile-major) slot layouts first; transposed to p-major at the
    # end so the device-side DMA reads DRAM contiguously per partition.
    root_core = np.zeros(seg_slots, dtype=np.int32)
    root_core[: s1 - s0] = root_full[s0:s1]

    gidx_core = np.zeros(e_slots, dtype=np.int32)
    slid_core = np.full(e_slots, -1.0, dtype=np.float32)
    for b in range(n_blocks):
        bs0 = s0 + b * P
        bs1 = min(bs0 + P, s1)
        if bs0 >= bs1:
            continue
        be0, be1 = cum[bs0], cum[bs1]
        n_b = be1 - be0
        if n_b > t_b * P:
            raise OverflowError(n_b)
        o0 = b * t_b * P
        gidx_core[o0 : o0 + n_b] = gid_sorted[be0:be1]
        slid_core[o0 : o0 + n_b] = (seg_sorted[be0:be1] - bs0).astype(np.float32)

    # slot (c, p) at linear c*P+p -> DRAM layout [p, c] (p-major rows).
    n_cols = e_slots // P
    gidx_core = np.ascontiguousarray(gidx_core.reshape(n_cols, P).T).ravel()
    slid_core = np.ascontiguousarray(slid_core.reshape(n_cols, P).T).ravel()
    root_core = np.ascontiguousarray(root_core.reshape(n_blocks, P).T).ravel()
    return root_core, gidx_core, slid_core


def _run(
    ast_np, wkt_s, wv_np, ast_to_pdg_key, ast_to_pdg_value,
    pdg_to_root_key, pdg_to_root_value, n_pdg,
    segs_per_core, n_blocks, t_b_min=T_B_MIN, trace=False,
):
    n_ast = ast_np.shape[0]

    order = np.argsort(ast_to_pdg_value, kind="stable")
    seg_sorted = np.asarray(ast_to_pdg_value)[order]
    gid_sorted = np.asarray(ast_to_pdg_key)[order].astype(np.int32)
    counts = np.bincount(seg_sorted, minlength=n_pdg)
    cum = np.concatenate([[0], np.cumsum(counts)]).astype(np.int64)

    root_full = np.zeros(n_pdg, dtype=np.int32)
    root_full[np.asarray(pdg_to_root_key)] = np.asarray(pdg_to_root_value)

    # Block capacity: elements per 128-seg block must fit t_b*128 slots.
    block_max = 0
    for c in range(N_CORES):
        s0 = c * segs_per_core
        s1 = min(s0 + segs_per_core, n_pdg)
        for b in range(n_blocks):
            bs0 = s0 + b * P
            bs1 = min(bs0 + P, s1)
            if bs0 < bs1:
                block_max = max(block_max, int(cum[bs1] - cum[bs0]))
    t_b = max(t_b_min, math.ceil(block_max / P))

    in_maps = []
    for c in range(N_CORES):
        root_core, gidx_core, slid_core = _prepare_core_inputs(
            c, n_pdg, segs_per_core, n_blocks, t_b,
            cum, seg_sorted, gid_sorted, root_full,
        )
        in_maps.append({
            "ast": ast_np,
            "wkt": wkt_s,
            "wv": wv_np,
            "gidx": gidx_core,
            "slid": slid_core,
            "root": root_core,
        })

    nc = _get_nc(n_ast, n_blocks, t_b)
    res = run_bass_kernel_spmd(nc, in_maps, list(range(N_CORES)), trace=trace)

    full = np.zeros((n_pdg, D), dtype=np.float32)
    for c in range(N_CORES):
        s0 = c * segs_per_core
        s1 = min(s0 + segs_per_core, n_pdg)
        full[s0:s1] = res.results[c]["out"][: s1 - s0]
    return full, res


def kernel(
    ast_nodes_encodings, Wk, Wv, ast_to_pdg_key, ast_to_pdg_value,
    pdg_to_root_key, pdg_to_root_value, nr_cfg_nodes,
):
    ast_np = np.ascontiguousarray(np.asarray(ast_nodes_encodings, dtype=np.float32))
    wk_np = np.asarray(Wk, dtype=np.float32)
    wv_np = np.ascontiguousarray(np.asarray(Wv, dtype=np.float32))
    scale = np.float32(1.0 / np.sqrt(ast_np.shape[1]))
    wkt_s = np.ascontiguousarray(wk_np.T * scale)

    n_pdg = int(nr_cfg_nodes)
    assert ast_np.shape == (N_AST_FULL, D) and n_pdg == N_PDG_FULL

    full, _ = _run(
        ast_np, wkt_s, wv_np,
        np.asarray(ast_to_pdg_key), np.asarray(ast_to_pdg_value),
        np.asarray(pdg_to_root_key), np.asarray(pdg_to_root_value),
        n_pdg, SEGS_PER_CORE_FULL, N_BLOCKS_FULL,
    )
    return full


# revision 19
# speedup vs baseline: 51.7849x; 51.7849x over previous
"""Trainium2 Bass kernel for CFGSubASTExpressionCombiner (segment-softmax
attention over sub-ASTs grouped by PDG node).

Contract: kernel(**inputs) takes FULL unsharded numpy inputs, returns the
FULL [N_PDG, D] output. Internally shards PDG segments across 8 NeuronCores
(contiguous segment ranges, elements sorted by segment on host), replicates
ast_nodes_encodings + weights, and runs one SPMD Bass program.

Math (per segment s with element set E_s, all on device):
    q_s   = ast[root(s)]
    qk_s  = q_s @ (Wk.T * scale)        (scale folded into host-passed WkT)
    S[e,s]= x_e . qk_s                  (dense per 128-seg block, via PE)
    P     = exp(S) * [seg(e)==s]        (masked exp == softmax numerator)
    [U|Z] = P.T @ [X|1]                 (segment sums via PE, PSUM accum)
    out_s = (U_s / max(Z_s,eps)) @ Wv
No max-subtraction: scores are ~N(0,1) (|s|<~6), exp is safe in fp32, and
softmax is shift-invariant so results match the reference to fp32 rounding.
"""

import math

import numpy as np

import concourse.bass as bass
import concourse.bacc as bacc
import concourse.mybir as mybir
import concourse.tile as tile
from concourse.bass_utils import run_bass_kernel_spmd
from concourse.masks import make_identity

P = 128
D = 256
N_CORES = 8

# Full-problem constants (hardcoded per contract).
N_AST_FULL = 500000
N_PDG_FULL = 50000
SEGS_PER_CORE_FULL = N_PDG_FULL // N_CORES          # 6250
N_BLOCKS_FULL = math.ceil(SEGS_PER_CORE_FULL / P)   # 49
T_B_MIN = 12                                        # element tiles per block

f32 = mybir.dt.float32
i32 = mybir.dt.int32


def _build_nc(n_ast, n_blocks, t_b, mode="full", batched=False, reps=1):
    """One SPMD NeuronCore program. Sizes fixed at build time.

    mode: "full" = real kernel; "gather" = gathers + tiny reduces only
    batched: one indirect gather per block instead of one per 128-row tile
    reps: repeat the whole block loop (differential timing only)
    """
    seg_slots = n_blocks * P
    e_slots = n_blocks * t_b * P

    nc = bacc.Bacc()
    ast = nc.declare_dram_parameter("ast", [n_ast, D], f32, isOutput=False)
    wkt = nc.declare_dram_parameter("wkt", [D, D], f32, isOutput=False)
    wv = nc.declare_dram_parameter("wv", [D, D], f32, isOutput=False)
    gidx = nc.declare_dram_parameter("gidx", [e_slots], i32, isOutput=False)
    slid = nc.declare_dram_parameter("slid", [e_slots], f32, isOutput=False)
    root = nc.declare_dram_parameter("root", [seg_slots], i32, isOutput=False)
    out = nc.declare_dram_parameter("out", [seg_slots, D], f32, isOutput=True)

    EXP = mybir.ActivationFunctionType.Exp
    COPY = mybir.ActivationFunctionType.Copy

    with tile.TileContext(nc) as tc:
        with (
            tc.tile_pool(name="const", bufs=1) as cpool,
            tc.tile_pool(name="blk", bufs=2) as bpool,
            tc.tile_pool(name="xp", bufs=3) as xpool,
            tc.tile_pool(name="pt", bufs=2, space="PSUM") as pt,
            tc.tile_pool(name="pu", bufs=2, space="PSUM") as pu,
        ):
            # Resident constants: Wk.T (pre-scaled) and Wv as two 128-row
            # K-chunks side by side; identity for PE transpose; iota row.
            wk2 = cpool.tile([P, 2 * D], f32)
            nc.sync.dma_start(out=wk2[:, 0:D], in_=wkt[0:P, :])
            nc.sync.dma_start(out=wk2[:, D : 2 * D], in_=wkt[P : 2 * P, :])
            wv2 = cpool.tile([P, 2 * D], f32)
            nc.sync.dma_start(out=wv2[:, 0:D], in_=wv[0:P, :])
            nc.sync.dma_start(out=wv2[:, D : 2 * D], in_=wv[P : 2 * P, :])
            ident = cpool.tile([P, P], f32)
            make_identity(nc, ident[:])
            iota_i = cpool.tile([P, P], i32)
            nc.gpsimd.iota(iota_i[:], pattern=[[1, P]], base=0, channel_multiplier=0)
            iota_f = cpool.tile([P, P], f32)
            nc.vector.tensor_copy(iota_f[:], iota_i[:])

            # All index arrays resident in SBUF, one DMA each: column c of
            # gx_all/sl_all is element-tile c = b*t_b + t; column b of
            # root_all is segment block b.
            n_cols = n_blocks * t_b
            gx_all = cpool.tile([P, n_cols], i32)
            nc.gpsimd.dma_start(
                out=gx_all[:], in_=gidx[:].rearrange("(p c) -> p c", c=n_cols)
            )
            sl_all = cpool.tile([P, n_cols], f32)
            nc.gpsimd.dma_start(
                out=sl_all[:], in_=slid[:].rearrange("(p c) -> p c", c=n_cols)
            )
            root_all = cpool.tile([P, n_blocks], i32)
            nc.gpsimd.dma_start(
                out=root_all[:], in_=root[:].rearrange("(p b) -> p b", b=n_blocks)
            )

            for _rep in range(reps):
              for b in range(n_blocks):
                # ---- segment side: q rows -> qk^T (d on partitions) ----
                q = bpool.tile([P, D], f32)
                nc.gpsimd.indirect_dma_start(
                    out=q[:],
                    out_offset=None,
                    in_=ast[:],
                    in_offset=bass.IndirectOffsetOnAxis(ap=root_all[:, b : b + 1], axis=0),
                )
                if mode == "gather":
                    acc = xpool.tile([P, t_b + 1], f32, tag="acc")
                    nc.vector.tensor_reduce(
                        acc[:, t_b : t_b + 1], q[:],
                        axis=mybir.AxisListType.X, op=mybir.AluOpType.max,
                    )
                    if batched:
                        x_blk = xpool.tile([P, t_b * D], f32, tag="xblk")
                        x3 = x_blk[:].rearrange("p (c d) -> p c d", d=D)
                        nc.gpsimd.indirect_dma_start(
                            out=x3,
                            out_offset=None,
                            in_=ast[:],
                            in_offset=bass.IndirectOffsetOnAxis(
                                ap=gx_all[:, b * t_b : (b + 1) * t_b], axis=0
                            ),
                        )
                        for t in range(t_b):
                            nc.vector.tensor_reduce(
                                acc[:, t : t + 1], x_blk[:, t * D : (t + 1) * D],
                                axis=mybir.AxisListType.X, op=mybir.AluOpType.max,
                            )
                    else:
                        for t in range(t_b):
                            c = b * t_b + t
                            x = xpool.tile([P, D], f32, tag="xg")
                            nc.gpsimd.indirect_dma_start(
                                out=x[:],
                                out_offset=None,
                                in_=ast[:],
                                in_offset=bass.IndirectOffsetOnAxis(
                                    ap=gx_all[:, c : c + 1], axis=0
                                ),
                            )
                            nc.vector.tensor_reduce(
                                acc[:, t : t + 1], x[:],
                                axis=mybir.AxisListType.X, op=mybir.AluOpType.max,
                            )
                    nc.sync.dma_start(
                        out=out[b * P : (b + 1) * P, 0 : t_b + 1], in_=acc[:]
                    )
                    continue
                qT_ps = pt.tile([P, D], f32, tag="tr")
                nc.tensor.transpose(qT_ps[:, 0:P], q[:, 0:P], ident[:])
                nc.tensor.transpose(qT_ps[:, P:D], q[:, P:D], ident[:])
                qT = bpool.tile([P, D], f32)
                nc.vector.tensor_copy(qT[:], qT_ps[:])

                qkT_ps = pt.tile([P, D], f32, tag="mm")
                for m in range(2):
                    for k in range(2):
                        nc.tensor.matmul(
                            qkT_ps[:, m * P : (m + 1) * P],
                            lhsT=wk2[:, k * D + m * P : k * D + (m + 1) * P],
                            rhs=qT[:, k * P : (k + 1) * P],
                            start=(k == 0),
                            stop=(k == 1),
                        )
                qkT = bpool.tile([P, D], f32)
                nc.vector.tensor_copy(qkT[:], qkT_ps[:])

                # ---- element side: accumulate [U | Z] over t_b tiles ----
                u_ps = pu.tile([P, D + 1], f32, tag="u")
                if batched:
                    x_blk = xpool.tile([P, t_b * (D + 1)], f32, tag="xblk")
                    x3 = x_blk[:].rearrange("p (c d) -> p c d", d=D + 1)
                    nc.vector.memset(x3[:, :, D : D + 1], 1.0)
                    nc.gpsimd.indirect_dma_start(
                        out=x3[:, :, 0:D],
                        out_offset=None,
                        in_=ast[:],
                        in_offset=bass.IndirectOffsetOnAxis(
                            ap=gx_all[:, b * t_b : (b + 1) * t_b], axis=0
                        ),
                    )
                for t in range(t_b):
                    c = b * t_b + t
                    if batched:
                        x = x_blk[:, t * (D + 1) : (t + 1) * (D + 1)]
                    else:
                        xt_ = xpool.tile([P, D + 1], f32)
                        x = xt_[:]
                        nc.vector.memset(x[:, D : D + 1], 1.0)
                        nc.gpsimd.indirect_dma_start(
                            out=x[:, 0:D],
                            out_offset=None,
                            in_=ast[:],
                            in_offset=bass.IndirectOffsetOnAxis(
                                ap=gx_all[:, c : c + 1], axis=0
                            ),
                        )
                    xT_ps = pt.tile([P, D], f32, tag="tr")
                    nc.tensor.transpose(xT_ps[:, 0:P], x[:, 0:P], ident[:])
                    nc.tensor.transpose(xT_ps[:, P:D], x[:, P:D], ident[:])
                    xT = xpool.tile([P, D], f32)
                    nc.vector.tensor_copy(xT[:], xT_ps[:])

                    s_ps = pt.tile([P, P], f32, tag="s")
                    for k in range(2):
                        nc.tensor.matmul(
                            s_ps[:],
                            lhsT=xT[:, k * P : (k + 1) * P],
                            rhs=qkT[:, k * P : (k + 1) * P],
                            start=(k == 0),
                            stop=(k == 1),
                        )
                    ptil = xpool.tile([P, P], f32)
                    nc.scalar.activation(ptil[:], s_ps[:], EXP)
                    ind = xpool.tile([P, P], f32)
                    nc.vector.tensor_scalar(
                        out=ind[:],
                        in0=iota_f[:],
                        scalar1=sl_all[:, c : c + 1],
                        scalar2=None,
                        op0=mybir.AluOpType.is_equal,
                    )
                    pmat = xpool.tile([P, P], f32)
                    nc.vector.tensor_mul(pmat[:], ptil[:], ind[:])
                    nc.tensor.matmul(
                        u_ps[:],
                        lhsT=pmat[:],
                        rhs=x[:],
                        start=(t == 0),
                        stop=(t == t_b - 1),
                    )

                # ---- finalize block: U/Z @ Wv ----
                z = bpool.tile([P, 1], f32)
                nc.vector.tensor_scalar_max(z[:], u_ps[:, D : D + 1], 1e-30)
                rz = bpool.tile([P, 1], f32)
                nc.vector.reciprocal(rz[:], z[:])
                up = bpool.tile([P, D], f32)
                nc.scalar.activation(up[:], u_ps[:, 0:D], COPY, scale=rz[:, :1])
                upT_ps = pt.tile([P, D], f32, tag="tr")
                nc.tensor.transpose(upT_ps[:, 0:P], up[:, 0:P], ident[:])
                nc.tensor.transpose(upT_ps[:, P:D], up[:, P:D], ident[:])
                upT = bpool.tile([P, D], f32)
                nc.vector.tensor_copy(upT[:], upT_ps[:])
                f_ps = pt.tile([P, D], f32, tag="mm")
                for k in range(2):
                    nc.tensor.matmul(
                        f_ps[:],
                        lhsT=upT[:, k * P : (k + 1) * P],
                        rhs=wv2[:, k * D : (k + 1) * D],
                        start=(k == 0),
                        stop=(k == 1),
                    )
                o = bpool.tile([P, D], f32)
                nc.vector.tensor_copy(o[:], f_ps[:])
                nc.sync.dma_start(out=out[b * P : (b + 1) * P, :], in_=o[:])
    nc.finalize()
    return nc


_NC_CACHE = {}


def _get_nc(n_ast, n_blocks, t_b, mode="full", batched=False, reps=1):
    key = (n_ast, n_blocks, t_b, mode, batched, reps)
    if key not in _NC_CACHE:
        _NC_CACHE[key] = _build_nc(
            n_ast, n_blocks, t_b, mode=mode, batched=batched, reps=reps
        )
    return _NC_CACHE[key]


def _prepare_core_inputs(
    c, n_pdg, segs_per_core, n_blocks, t_b, cum, seg_sorted, gid_sorted, root_full
):
    """Padded per-core index arrays for core c's contiguous segment range."""
    seg_slots = n_blocks * P
    e_slots = n_blocks * t_b * P
    s0 = c * segs_per_core
    s1 = min(s0 + segs_per_core, s0 + seg_slots, n_pdg)

    # Linear (tile-major) slot layouts first; transposed to p-major at the
    # end so the device-side DMA reads DRAM contiguously per partition.
    root_core = np.zeros(seg_slots, dtype=np.int32)
    root_core[: s1 - s0] = root_full[s0:s1]

    gidx_core = np.zeros(e_slots, dtype=np.int32)
    slid_core = np.full(e_slots, -1.0, dtype=np.float32)
    for b in range(n_blocks):
        bs0 = s0 + b * P
        bs1 = min(bs0 + P, s1)
        if bs0 >= bs1:
            continue
        be0, be1 = cum[bs0], cum[bs1]
        n_b = be1 - be0
        if n_b > t_b * P:
            raise OverflowError(n_b)
        o0 = b * t_b * P
        gidx_core[o0 : o0 + n_b] = gid_sorted[be0:be1]
        slid_core[o0 : o0 + n_b] = (seg_sorted[be0:be1] - bs0).astype(np.float32)

    # slot (c, p) at linear c*P+p -> DRAM layout [p, c] (p-major rows).
    n_cols = e_slots // P
    gidx_core = np.ascontiguousarray(gidx_core.reshape(n_cols, P).T).ravel()
    slid_core = np.ascontiguousarray(slid_core.reshape(n_cols, P).T).ravel()
    root_core = np.ascontiguousarray(root_core.reshape(n_blocks, P).T).ravel()
    return root_core, gidx_core, slid_core


def _run(
    ast_np, wkt_s, wv_np, ast_to_pdg_key, ast_to_pdg_value,
    pdg_to_root_key, pdg_to_root_value, n_pdg,
    segs_per_core, n_blocks, t_b_min=T_B_MIN, trace=False, batched=False,
):
    n_ast = ast_np.shape[0]

    order = np.argsort(ast_to_pdg_value, kind="stable")
    seg_sorted = np.asarray(ast_to_pdg_value)[order]
    gid_sorted = np.asarray(ast_to_pdg_key)[order].astype(np.int32)
    counts = np.bincount(seg_sorted, minlength=n_pdg)
    cum = np.concatenate([[0], np.cumsum(counts)]).astype(np.int64)

    root_full = np.zeros(n_pdg, dtype=np.int32)
    root_full[np.asarray(pdg_to_root_key)] = np.asarray(pdg_to_root_value)

    # Block capacity: elements per 128-seg block must fit t_b*128 slots.
    block_max = 0
    for c in range(N_CORES):
        s0 = c * segs_per_core
        s1 = min(s0 + segs_per_core, n_pdg)
        for b in range(n_blocks):
            bs0 = s0 + b * P
            bs1 = min(bs0 + P, s1)
            if bs0 < bs1:
                block_max = max(block_max, int(cum[bs1] - cum[bs0]))
    t_b = max(t_b_min, math.ceil(block_max / P))

    in_maps = []
    for c in range(N_CORES):
        root_core, gidx_core, slid_core = _prepare_core_inputs(
            c, n_pdg, segs_per_core, n_blocks, t_b,
            cum, seg_sorted, gid_sorted, root_full,
        )
        in_maps.append({
            "ast": ast_np,
            "wkt": wkt_s,
            "wv": wv_np,
            "gidx": gidx_core,
            "slid": slid_core,
            "root": root_core,
        })

    nc = _get_nc(n_ast, n_blocks, t_b, batched=batched)
    res = run_bass_kernel_spmd(nc, in_maps, list(range(N_CORES)), trace=trace)

    full = np.zeros((n_pdg, D), dtype=np.float32)
    for c in range(N_CORES):
        s0 = c * segs_per_core
        s1 = min(s0 + segs_per_core, n_pdg)
        full[s0:s1] = res.results[c]["out"][: s1 - s0]
    return full, res


def kernel(
    ast_nodes_encodings, Wk, Wv, ast_to_pdg_key, ast_to_pdg_value,
    pdg_to_root_key, pdg_to_root_value, nr_cfg_nodes,
):
    ast_np = np.ascontiguousarray(np.asarray(ast_nodes_encodings, dtype=np.float32))
    wk_np = np.asarray(Wk, dtype=np.float32)
    wv_np = np.ascontiguousarray(np.asarray(Wv, dtype=np.float32))
    scale = np.float32(1.0 / np.sqrt(ast_np.shape[1]))
    wkt_s = np.ascontiguousarray(wk_np.T * scale)

    n_pdg = int(nr_cfg_nodes)
    assert ast_np.shape == (N_AST_FULL, D) and n_pdg == N_PDG_FULL

    full, _ = _run(
        ast_np, wkt_s, wv_np,
        np.asarray(ast_to_pdg_key), np.asarray(ast_to_pdg_value),
        np.asarray(pdg_to_root_key), np.asarray(pdg_to_root_value),
        n_pdg, SEGS_PER_CORE_FULL, N_BLOCKS_FULL,
    )
    return full
